# revision 1
# baseline (speedup 1.0000x reference)
"""Trainium2 Bass kernel for nn_Attention1D (B=4, L=4096, C=64).

reference:
    Q = x@Wq + bq ; K = x@Wk + bk ; V = x@Wv + bv          (per batch b)
    s = Q @ K.T / sqrt(C)                                   [L_q, L_k]
    attn = softmax(s, axis=q)      # normalize over QUERY axis
    out = attn @ V + x

Sharding: 8 cores = 4 batches x 2 key-shards (k in [0,2048) / [2048,4096)).
The softmax normalizes over q, which is NOT sharded, so each core's softmax
is fully local:
    Z[k]   = sum_q exp(s[q,k])
    out_qf = sum_k exp(s[q,k]) * (V[k,f]/Z[k])
and the two k-shards' partial outputs simply ADD. The host sums the pair
and adds the residual x (the residual dominates the output, which also
makes the attention path tolerant of bf16).

Layout: channel-major (c on partitions) everywhere, so scores come out
transposed sT[k, q] with the softmax axis on the free dim:
    sT chunk = matmul(lhsT=KT[c,k-tile(128)], rhs=QT[c,q-chunk(512)])  f32r
    exp+Z    = one ScalarE pass per [128,1024] PSUM chunk (accum_out)
    out      = PSUM-accumulated over 16 k-tiles:
               matmul(acc[qc], lhsT=ET[k,qc*128:+128](bf16), rhs=GV[k,f])
Host pre-transposes x and appends a ones-row so biases ride inside the
weights (contract dim 65); 1/sqrt(C) is folded into Wq. Q/K path runs in
float32r (fp32 data, full-rate PE mode, ~tf32 precision) because softmax
exponentiates absolute score errors: bf16 Q/K costs 1e-2 rel error, f32r
1e-4. V/ET stay bf16. No max-subtraction (|s| <= ~9, exp is safe in fp32).

PSUM (8 banks): 2 x [128,1024]f32 score slots (4 banks, double-buffered,
evacuated directly by the ACT exp) + [128,32,64]f32 out accumulator
(4 banks). matmul start=True clears has_written for the WHOLE bank, so only
the first accumulator chunk-MM per bank sets it.

A ~7us dummy-matmul warmup burst runs during the input DMAs: the PE's HAM
clock gate only reaches 2.4 GHz after ~3.4us of *continuous* busy; without
it the whole kernel runs at 1.2 GHz.
"""

import numpy as np
import ml_dtypes  # noqa: F401  (np bf16 support registered on import)

B, L, C = 4, 4096, 64
NCORES = 8
KSH = L // 2          # k columns per core: 2048
NKT = KSH // 128      # 16 k-tiles per core
NQC = L // 128        # 32 q-chunks of 128
NQ5 = L // 512        # 8 q-chunks of 512

_cache = {}


def _patch_ldw_opt():
    # walrus is invoked with --enable-ldw-opt=false hardcoded; redundant
    # LDWEIGHTS (8 same-weight score matmuls per k-tile) cost ~40us/core.
    import concourse.bass_utils as bu
    if getattr(bu, "_ldw_patched", False):
        return
    orig = bu.run_command

    def run_command_ldw(cmd, *a, **kw):
        if isinstance(cmd, list):
            cmd = [c.replace("--enable-ldw-opt=false", "--enable-ldw-opt=true")
                   if isinstance(c, str) else c for c in cmd]
        return orig(cmd, *a, **kw)

    bu.run_command = run_command_ldw
    bu._ldw_patched = True


def _build():
    import concourse.bacc as bacc
    import concourse.mybir as mybir
    import concourse.tile as tile
    from concourse.bass import _add_dep_helper


    bf16 = mybir.dt.bfloat16
    f32 = mybir.dt.float32
    f32r = mybir.dt.float32r
    i32 = mybir.dt.int32
    AF = mybir.ActivationFunctionType
    AX = mybir.AxisListType

    nc = bacc.Bacc("TRN2", target_bir_lowering=False, debug=False)

    xt_d = nc.dram_tensor("xt", [C + 1, L], f32r, kind="ExternalInput")
    xk_d = nc.dram_tensor("xk", [C + 1, KSH], f32r, kind="ExternalInput")
    wq_d = nc.dram_tensor("wq", [C + 1, 2 * C], f32r, kind="ExternalInput")
    wk_d = nc.dram_tensor("wk", [C + 1, 2 * C], f32r, kind="ExternalInput")
    wv_d = nc.dram_tensor("wv", [C + 1, C], f32r, kind="ExternalInput")
    o_d = nc.dram_tensor("o", [L, C], f32, kind="ExternalOutput")

    with tile.TileContext(nc) as tc:
        with (
            tc.tile_pool(name="consts", bufs=1) as consts,
            tc.tile_pool(name="sb", bufs=1) as sb,
            tc.tile_pool(name="etp", bufs=4) as etp,
            tc.tile_pool(name="gvp", bufs=4) as gvp,
            tc.tile_pool(name="zpp", bufs=6) as zpp,
            tc.tile_pool(name="scp", bufs=2, space="PSUM") as scp,
            tc.tile_pool(name="accp", bufs=1, space="PSUM") as accp,
        ):
            # --- HAM warmup: dense dummy matmuls while the DMAs stream in ---
            wu = consts.tile([128, 512], bf16)
            nc.vector.memset(wu, 0.0)
            for _ in range(10):
                ps = scp.tile([128, 512], f32, tag="s")
                nc.tensor.matmul(ps, lhsT=wu[:, 0:128], rhs=wu,
                                 start=True, stop=True)

            wq_s = consts.tile([C + 1, 2 * C], f32r)
            wk_s = consts.tile([C + 1, 2 * C], f32r)
            wv_s = consts.tile([C + 1, C], f32r)
            nc.sync.dma_start(out=wq_s, in_=wq_d.ap())
            nc.sync.dma_start(out=wk_s, in_=wk_d.ap())
            nc.sync.dma_start(out=wv_s, in_=wv_d.ap())

            # per-512-chunk input tiles -> precise DMA->matmul dependencies
            xt_c = []
            for c in range(NQ5):
                t = sb.tile([C + 1, 512], f32r, tag=f"xt{c}")
                nc.sync.dma_start(out=t, in_=xt_d.ap()[:, c * 512:(c + 1) * 512])
                xt_c.append(t)
            xk_c = []
            for c in range(KSH // 512):
                t = sb.tile([C + 1, 512], f32r, tag=f"xk{c}")
                nc.sync.dma_start(out=t, in_=xk_d.ap()[:, c * 512:(c + 1) * 512])
                xk_c.append(t)

            # QT/KT chunks [128, 512]: rows 0-63 and 64-127 hold the SAME
            # values (weights doubled host-side) so score matmuls can be
            # row-packed two k-tiles at a time via tile_position.
            qt_c = []
            for c in range(NQ5):
                ps = scp.tile([128, 512], f32, tag="s")
                nc.tensor.matmul(ps, lhsT=wq_s, rhs=xt_c[c],
                                 start=True, stop=True)
                t = sb.tile([128, 512], f32r, tag=f"qt{c}")
                nc.vector.tensor_copy(out=t, in_=ps)
                qt_c.append(t)
            kt_c = []
            for c in range(KSH // 512):
                ps = scp.tile([128, 512], f32, tag="s")
                nc.tensor.matmul(ps, lhsT=wk_s, rhs=xk_c[c],
                                 start=True, stop=True)
                t = sb.tile([128, 512], f32r, tag=f"kt{c}")
                nc.vector.tensor_copy(out=t, in_=ps)
                kt_c.append(t)

            v_ts = []  # V [k(128), f] per k-tile, bf16
            for kt in range(NKT):
                vps = scp.tile([128, C], f32, tag="s")
                nc.tensor.matmul(
                    vps,
                    lhsT=xk_c[kt // 4][:, (kt % 4) * 128:(kt % 4 + 1) * 128],
                    rhs=wv_s, start=True, stop=True,
                )
                v_t = sb.tile([128, C], bf16, tag=f"v{kt}")
                nc.vector.tensor_copy(out=v_t, in_=vps)
                v_ts.append(v_t)

            # --- main loop over k-tiles ---
            acc = accp.tile([128, NQC, C], f32)   # 4 PSUM banks, whole loop
            prev = None

            def emit_av_group(p, c2):
                # 8 AV chunk-MMs of the previous k-tile, interleaved between
                # score chunks to keep the PE dense.
                et_p, gv_p, kt_p = p
                for qc in range(c2 * 8, c2 * 8 + 8):
                    # start=True clears has_written for the WHOLE bank: only
                    # the first chunk-MM per bank may set it; later chunks
                    # overwrite-where-unset, which sets their own bits.
                    nc.tensor.matmul(
                        acc[:, qc, :],
                        lhsT=et_p[:, qc * 128:(qc + 1) * 128],
                        rhs=gv_p,
                        start=(kt_p == 0 and qc % 8 == 0),
                        stop=(kt_p == NKT - 1),
                        skip_group_check=True,
                    )

            # k-tiles processed in PAIRS: the score matmuls contract only 64
            # channels, so tile A runs in PE rows 0-63 and tile B in rows
            # 64-127 concurrently (tile_position row packing) -> ~2x.
            for kp in range(NKT // 2):
                kA, kB = 2 * kp, 2 * kp + 1
                etA = etp.tile([128, L], bf16, tag="etA")
                etB = etp.tile([128, L], bf16, tag="etB")
                zpA = zpp.tile([128, 4], f32, tag="zpA")
                zpB = zpp.tile([128, 4], f32, tag="zpB")
                lA = kt_c[kA // 4][0:C, (kA % 4) * 128:(kA % 4 + 1) * 128]
                lB = kt_c[kB // 4][C:128, (kB % 4) * 128:(kB % 4 + 1) * 128]
                for c2 in range(4):
                    stA = scp.tile([128, 1024], f32, tag="s")
                    stB = scp.tile([128, 1024], f32, tag="s")
                    last = None
                    for h in range(2):
                        rhs = qt_c[c2 * 2 + h]
                        ma = nc.tensor.matmul(
                            stA[:, h * 512:(h + 1) * 512], lhsT=lA,
                            rhs=rhs[0:C, :], tile_position=(0, 0),
                            start=True, stop=True,
                        )
                        mb = nc.tensor.matmul(
                            stB[:, h * 512:(h + 1) * 512], lhsT=lB,
                            rhs=rhs[C:128, :], tile_position=(C, 0),
                            start=True, stop=True,
                        )
                        # keep the A/B pair adjacent in the static PE order so
                        # the row-packed halves co-issue (scheduler otherwise
                        # sometimes emits [B,B,A,A], serializing the pair)
                        if last is not None:
                            _add_dep_helper(ma.ins, last.ins, sync=False,
                                            reason="pair order")
                        _add_dep_helper(mb.ins, ma.ins, sync=False,
                                        reason="pair order")
                        last = mb
                    nc.scalar.activation(
                        out=etA[:, c2 * 1024:(c2 + 1) * 1024], in_=stA,
                        func=AF.Exp, accum_out=zpA[:, c2:c2 + 1],
                    )
                    nc.scalar.activation(
                        out=etB[:, c2 * 1024:(c2 + 1) * 1024], in_=stB,
                        func=AF.Exp, accum_out=zpB[:, c2:c2 + 1],
                    )
                    if prev is not None:
                        emit_av_group(prev[0], c2)
                        emit_av_group(prev[1], c2)
                gvs = []
                for kt, zp, vv in ((kA, zpA, v_ts[kA]), (kB, zpB, v_ts[kB])):
                    z = zpp.tile([128, 1], f32, tag=f"z{kt % 2}")
                    nc.vector.reduce_sum(out=z, in_=zp, axis=AX.X)
                    rz = zpp.tile([128, 1], f32, tag=f"rz{kt % 2}")
                    nc.vector.reciprocal(out=rz, in_=z)
                    gv = gvp.tile([128, C], bf16, tag=f"gv{kt % 2}")
                    nc.vector.tensor_scalar_mul(gv, vv, rz)
                    gvs.append(gv)
                prev = ((etA, gvs[0], kA), (etB, gvs[1], kB))
            # final pair's AV drain, interleaved with the per-bank
            # evacuation + store so the tail overlaps the remaining AV work
            o_ap = o_d.ap()
            for g in range(4):
                emit_av_group(prev[0], g)
                emit_av_group(prev[1], g)
                ob = sb.tile([128, 8, C], f32, tag=f"ob{g}")
                nc.vector.tensor_copy(out=ob, in_=acc[:, g * 8:(g + 1) * 8, :])
                nc.sync.dma_start(
                    out=o_ap[g * 1024:(g + 1) * 1024, :].rearrange(
                        "(t p) f -> p t f", p=128
                    ),
                    in_=ob,
                )

    nc.compile()
    return nc


def _get_nc():
    if "nc" not in _cache:
        _cache["nc"] = _build()
    return _cache["nc"]


def _in_maps(x, Wq, bq, Wk, bk, Wv, bv):
    s = 1.0 / np.sqrt(np.float32(C))
    wq1 = (np.concatenate([Wq, bq[None, :]], 0) * s).astype(np.float32)
    wq1 = np.concatenate([wq1, wq1], 1)          # doubled -> replicated QT
    wk1 = np.concatenate([Wk, bk[None, :]], 0).astype(np.float32)
    wk1 = np.concatenate([wk1, wk1], 1)
    wv1 = np.concatenate([Wv, bv[None, :]], 0).astype(np.float32)
    maps = []
    for core in range(NCORES):
        b, half = core // 2, core % 2
        x1t = np.ascontiguousarray(np.concatenate(
            [x[b], np.ones((L, 1), np.float32)], 1
        ).T.astype(np.float32))              # [65, L]
        xk = np.ascontiguousarray(x1t[:, half * KSH:(half + 1) * KSH])
        maps.append({
            "xt": x1t,
            "xk": xk,
            "wq": wq1, "wk": wk1, "wv": wv1,
        })
    return maps


def _run(x, Wq, bq, Wk, bk, Wv, bv, trace=False):
    from concourse.bass_utils import run_bass_kernel_spmd

    nc = _get_nc()
    maps = _in_maps(x, Wq, bq, Wk, bk, Wv, bv)
    res = run_bass_kernel_spmd(
        nc, maps, core_ids=list(range(NCORES)), trace=trace
    )
    outs = [r["o"].astype(np.float32) for r in res.results]
    full = np.empty((B, L, C), np.float32)
    for b in range(B):
        full[b] = outs[2 * b] + outs[2 * b + 1] + x[b]
    return full, res


def kernel(x, Wq, bq, Wk, bk, Wv, bv):
    x = np.asarray(x, np.float32)
    full, _ = _run(
        x,
        np.asarray(Wq, np.float32), np.asarray(bq, np.float32),
        np.asarray(Wk, np.float32), np.asarray(bk, np.float32),
        np.asarray(Wv, np.float32), np.asarray(bv, np.float32),
    )
    return full



# revision 6
# speedup vs baseline: 1.0780x; 1.0780x over previous
"""Trainium2 Bass kernel for nn_Attention1D (B=4, L=4096, C=64).

reference:
    Q = x@Wq + bq ; K = x@Wk + bk ; V = x@Wv + bv          (per batch b)
    s = Q @ K.T / sqrt(C)                                   [L_q, L_k]
    attn = softmax(s, axis=q)      # normalize over QUERY axis
    out = attn @ V + x

Sharding: 8 cores = 4 batches x 2 key-shards (k in [0,2048) / [2048,4096)).
The softmax normalizes over q, which is NOT sharded, so each core's softmax
is fully local:
    Z[k]   = sum_q exp(s[q,k])
    out_qf = sum_k exp(s[q,k]) * (V[k,f]/Z[k])
and the two k-shards' partial outputs simply ADD. The host sums the pair
and adds the residual x (the residual dominates the output, which also
makes the attention path tolerant of bf16).

Roofline: the ScalarE (ACT) exp of 2048x4096 = 8.4M score elements per
core is the binding engine (~1 elem/cycle/lane @1.2GHz + 352-cycle
instruction overhead -> ~75-85us at 1024-col chunks). Everything else is
structured to keep ACT 100% busy:
  - scores in BF16 (not f32r): fp32 rhs streams at ~2 cycles/column and
    its FP32-HI weight loads disable FWL for the following (AV) LDWEIGHTS.
    bf16 halves score-MM time and keeps FWL on. Costs ~1e-2 rel err
    (amplified through exp), well under the 2e-2 gate.
  - a dummy exp at t=0 forces the ~1.3us ACT table load during the DMAs.
  - Q/K/V projections stay f32r (accurate) and are emitted just-in-time:
    only KT tiles 0-7 + QT chunk 0 precede the main loop; the rest
    interleave into PE slack between score groups.
  - PSUM: 2 x [128,1024]f32 score slots (4 banks; per group, slot A holds
    k-tile A's chunk and slot B k-tile B's) + [128,32,64]f32 out
    accumulator (4 banks). matmul start=True clears has_written for the
    WHOLE bank, so only the first accumulator chunk-MM per bank sets it.

Layout: channel-major (c on partitions), scores transposed sT[k, q] with
the softmax axis on the free dim:
    sT chunk = matmul(lhsT=KT[c(64),k-tile(128)], rhs=QT[c,q-chunk(1024)])
               two k-tiles row-packed via tile_position (A rows 0-63,
               B rows 64-127; QT/KT rows duplicated host-side)
    exp+Z    = one ScalarE pass per [128,1024] PSUM chunk (accum_out)
    out      = PSUM-accumulated over 16 k-tiles:
               matmul(acc[qc], lhsT=ET[k,qc*128:+128](bf16), rhs=GV[k,f])
Host pre-transposes x and appends a ones-row so biases ride inside the
weights (contract dim 65); 1/sqrt(C) is folded into Wq.
No max-subtraction (|s| <= ~9, exp is safe in fp32).
"""

import numpy as np
import ml_dtypes  # noqa: F401  (np bf16 support registered on import)

B, L, C = 4, 4096, 64
NCORES = 8
KSH = L // 2          # k columns per core: 2048
NKT = KSH // 128      # 16 k-tiles per core
NQC = L // 128        # 32 q-chunks of 128
NQ1 = L // 1024       # 4 q-chunks of 1024

_cache = {}


def _patch_ldw_opt():
    # walrus is invoked with --enable-ldw-opt=false hardcoded; redundant
    # LDWEIGHTS (4 same-weight score matmuls per k-tile) are pure waste.
    import concourse.bass_utils as bu
    if getattr(bu, "_ldw_patched", False):
        return
    orig = bu.run_command

    def run_command_ldw(cmd, *a, **kw):
        if isinstance(cmd, list):
            cmd = [c.replace("--enable-ldw-opt=false", "--enable-ldw-opt=true")
                   if isinstance(c, str) else c for c in cmd]
        return orig(cmd, *a, **kw)

    bu.run_command = run_command_ldw
    bu._ldw_patched = True


def _build():
    # NOTE: _patch_ldw_opt (--enable-ldw-opt=true) fails walrus codegen here:
    # "InstLdweights is not compatible with LDW optimization" on the
    # tile_position score LDWs. Left disabled.
    import concourse.bacc as bacc
    import concourse.mybir as mybir
    import concourse.tile as tile
    from concourse.bass import _add_dep_helper

    bf16 = mybir.dt.bfloat16
    f32 = mybir.dt.float32
    f32r = mybir.dt.float32r
    AF = mybir.ActivationFunctionType
    AX = mybir.AxisListType

    nc = bacc.Bacc("TRN2", target_bir_lowering=False, debug=False)

    xt_d = nc.dram_tensor("xt", [C + 1, L], f32r, kind="ExternalInput")
    xk_d = nc.dram_tensor("xk", [C + 1, KSH], f32r, kind="ExternalInput")
    wq_d = nc.dram_tensor("wq", [C + 1, 2 * C], f32r, kind="ExternalInput")
    wk_d = nc.dram_tensor("wk", [C + 1, 2 * C], f32r, kind="ExternalInput")
    wv_d = nc.dram_tensor("wv", [C + 1, C], f32r, kind="ExternalInput")
    o_d = nc.dram_tensor("o", [L, C], f32, kind="ExternalOutput")

    with tile.TileContext(nc) as tc:
        with (
            tc.tile_pool(name="consts", bufs=1) as consts,
            tc.tile_pool(name="sb", bufs=1) as sb,
            tc.tile_pool(name="etp", bufs=4) as etp,
            tc.tile_pool(name="gvp", bufs=4) as gvp,
            tc.tile_pool(name="zpp", bufs=6) as zpp,
            tc.tile_pool(name="scp", bufs=2, space="PSUM") as scp,
            tc.tile_pool(name="accp", bufs=1, space="PSUM") as accp,
        ):
            # --- ACT table warmer: walrus inserts the ~1.3us
            # PSEUDO_LOAD_ACT_FUNC_SET before this dummy exp, so the table
            # is resident long before the first real score chunk. ---
            jk = consts.tile([128, 1], f32)
            nc.vector.memset(jk, 0.0)
            jko = consts.tile([128, 1], f32)
            nc.scalar.activation(out=jko, in_=jk, func=AF.Exp)

            # --- HAM warmup: dense dummy matmuls while the DMAs stream in ---
            wu = consts.tile([128, 512], bf16)
            nc.vector.memset(wu, 0.0)
            for _ in range(8):
                ps = scp.tile([128, 1024], f32, tag="s")
                nc.tensor.matmul(ps[:, 0:512], lhsT=wu[:, 0:128], rhs=wu,
                                 start=True, stop=True)

            # --- input DMAs, critical-path order (Sync queue serializes
            # issue at ~0.8us each): K path first, then Q chunk 0. ---
            wk_s = consts.tile([C + 1, 2 * C], f32r)
            wq_s = consts.tile([C + 1, 2 * C], f32r)
            wv_s = consts.tile([C + 1, C], f32r)
            nc.sync.dma_start(out=wk_s, in_=wk_d.ap())
            xk_c = []
            for c in range(KSH // 1024):
                t = sb.tile([C + 1, 1024], f32r, tag=f"xk{c}")
                nc.sync.dma_start(out=t, in_=xk_d.ap()[:, c * 1024:(c + 1) * 1024])
                xk_c.append(t)
            nc.sync.dma_start(out=wq_s, in_=wq_d.ap())
            xt_c = []
            for c in range(NQ1):
                t = sb.tile([C + 1, 1024], f32r, tag=f"xt{c}")
                nc.sync.dma_start(out=t, in_=xt_d.ap()[:, c * 1024:(c + 1) * 1024])
                xt_c.append(t)
                if c == 0:
                    nc.sync.dma_start(out=wv_s, in_=wv_d.ap())

            # --- projections (f32r, accurate). QT/KT rows 0-63 and 64-127
            # hold the SAME values (weights doubled host-side) so score
            # matmuls row-pack two k-tiles via tile_position. Emitted
            # just-in-time: prologue does KT chunk 0 (k-tiles 0-7) and QT
            # chunk 0; the rest interleave into the main loop. ---
            kt_c = [sb.tile([128, 1024], bf16, tag=f"kt{c}", name=f"kt{c}")
                    for c in range(2)]
            qt_c = [sb.tile([128, 1024], bf16, tag=f"qt{c}", name=f"qt{c}")
                    for c in range(NQ1)]
            v_ts = [sb.tile([128, C], bf16, tag=f"v{kt}", name=f"v{kt}")
                    for kt in range(NKT)]

            def emit_kt(c):
                # fp32 moving-operand limit is 512 columns -> two MMs per
                # 1024 chunk (each covers exactly one PSUM bank).
                ps = scp.tile([128, 1024], f32, tag="s")
                for h in range(2):
                    nc.tensor.matmul(ps[:, h * 512:(h + 1) * 512], lhsT=wk_s,
                                     rhs=xk_c[c][:, h * 512:(h + 1) * 512],
                                     start=True, stop=True)
                nc.vector.tensor_copy(out=kt_c[c], in_=ps)

            def emit_qt(c):
                ps = scp.tile([128, 1024], f32, tag="s")
                for h in range(2):
                    nc.tensor.matmul(ps[:, h * 512:(h + 1) * 512], lhsT=wq_s,
                                     rhs=xt_c[c][:, h * 512:(h + 1) * 512],
                                     start=True, stop=True)
                nc.vector.tensor_copy(out=qt_c[c], in_=ps)

            def emit_v(kt):
                vps = scp.tile([128, 1024], f32, tag="s")
                nc.tensor.matmul(
                    vps[:, 0:C],
                    lhsT=xk_c[kt // 8][:, (kt % 8) * 128:(kt % 8 + 1) * 128],
                    rhs=wv_s, start=True, stop=True,
                )
                nc.vector.tensor_copy(out=v_ts[kt], in_=vps[:, 0:C])

            emit_kt(0)
            emit_qt(0)
            emit_v(0)
            emit_v(1)

            # deferred projection work, drained 1 item per (pair, group)
            deferred = [lambda c=c: emit_qt(c) for c in range(1, NQ1)]
            deferred.append(lambda: emit_kt(1))
            deferred += [lambda k=k: emit_v(k) for k in range(2, NKT)]

            # --- main loop over k-tile pairs ---
            acc = accp.tile([128, NQC, C], f32)   # 4 PSUM banks, whole loop
            prev = None

            def emit_av_group(p, c2):
                # 8 AV chunk-MMs of the previous k-tile, interleaved between
                # score chunks to keep the PE dense.
                et_p, gv_p, kt_p = p
                for qc in range(c2 * 8, c2 * 8 + 8):
                    # start=True clears has_written for the WHOLE bank: only
                    # the first chunk-MM per bank may set it; later chunks
                    # overwrite-where-unset, which sets their own bits.
                    nc.tensor.matmul(
                        acc[:, qc, :],
                        lhsT=et_p[:, qc * 128:(qc + 1) * 128],
                        rhs=gv_p,
                        start=(kt_p == 0 and qc % 8 == 0),
                        stop=(kt_p == NKT - 1),
                        skip_group_check=True,
                    )

            # k-tiles processed in PAIRS: the score matmuls contract only 64
            # channels, so tile A runs in PE rows 0-63 and tile B in rows
            # 64-127 concurrently (tile_position row packing) -> ~2x.
            last = None
            for kp in range(NKT // 2):
                kA, kB = 2 * kp, 2 * kp + 1
                etA = etp.tile([128, L], bf16, tag="etA")
                etB = etp.tile([128, L], bf16, tag="etB")
                zpA = zpp.tile([128, 4], f32, tag="zpA")
                zpB = zpp.tile([128, 4], f32, tag="zpB")
                lA = kt_c[kA // 8][0:C, (kA % 8) * 128:(kA % 8 + 1) * 128]
                lB = kt_c[kB // 8][C:128, (kB % 8) * 128:(kB % 8 + 1) * 128]
                for c2 in range(4):
                    stA = scp.tile([128, 1024], f32, tag="s")
                    stB = scp.tile([128, 1024], f32, tag="s")
                    rhs = qt_c[c2]
                    for h in range(2):
                        hs = slice(h * 512, (h + 1) * 512)
                        ma = nc.tensor.matmul(
                            stA[:, hs], lhsT=lA, rhs=rhs[0:C, hs],
                            tile_position=(0, 0), start=True, stop=True,
                        )
                        mb = nc.tensor.matmul(
                            stB[:, hs], lhsT=lB, rhs=rhs[C:128, hs],
                            tile_position=(C, 0), start=True, stop=True,
                        )
                        # keep the A/B pair adjacent in the static PE order
                        # so the row-packed halves co-issue (scheduler
                        # otherwise sometimes emits [B,B,A,A], serializing)
                        if last is not None:
                            _add_dep_helper(ma.ins, last.ins, sync=False,
                                            reason="pair order")
                        _add_dep_helper(mb.ins, ma.ins, sync=False,
                                        reason="pair order")
                        last = mb
                    nc.scalar.activation(
                        out=etA[:, c2 * 1024:(c2 + 1) * 1024], in_=stA,
                        func=AF.Exp, accum_out=zpA[:, c2:c2 + 1],
                    )
                    nc.scalar.activation(
                        out=etB[:, c2 * 1024:(c2 + 1) * 1024], in_=stB,
                        func=AF.Exp, accum_out=zpB[:, c2:c2 + 1],
                    )
                    if prev is not None:
                        emit_av_group(prev[0], c2)
                        emit_av_group(prev[1], c2)
                    if deferred:
                        deferred.pop(0)()
                gvs = []
                for kt, zp, vv in ((kA, zpA, v_ts[kA]), (kB, zpB, v_ts[kB])):
                    z = zpp.tile([128, 1], f32, tag=f"z{kt % 2}")
                    nc.vector.reduce_sum(out=z, in_=zp, axis=AX.X)
                    rz = zpp.tile([128, 1], f32, tag=f"rz{kt % 2}")
                    nc.vector.reciprocal(out=rz, in_=z)
                    gv = gvp.tile([128, C], bf16, tag=f"gv{kt % 2}")
                    nc.vector.tensor_scalar_mul(gv, vv, rz)
                    gvs.append(gv)
                prev = ((etA, gvs[0], kA), (etB, gvs[1], kB))
            # final pair's AV drain, interleaved with the per-bank
            # evacuation + store so the tail overlaps the remaining AV work
            o_ap = o_d.ap()
            for g in range(4):
                emit_av_group(prev[0], g)
                emit_av_group(prev[1], g)
                ob = sb.tile([128, 8, C], f32, tag=f"ob{g}")
                nc.vector.tensor_copy(out=ob, in_=acc[:, g * 8:(g + 1) * 8, :])
                nc.sync.dma_start(
                    out=o_ap[g * 1024:(g + 1) * 1024, :].rearrange(
                        "(t p) f -> p t f", p=128
                    ),
                    in_=ob,
                )

    nc.compile()
    return nc


def _get_nc():
    if "nc" not in _cache:
        _cache["nc"] = _build()
    return _cache["nc"]


def _in_maps(x, Wq, bq, Wk, bk, Wv, bv):
    s = 1.0 / np.sqrt(np.float32(C))
    wq1 = (np.concatenate([Wq, bq[None, :]], 0) * s).astype(np.float32)
    wq1 = np.concatenate([wq1, wq1], 1)          # doubled -> replicated QT
    wk1 = np.concatenate([Wk, bk[None, :]], 0).astype(np.float32)
    wk1 = np.concatenate([wk1, wk1], 1)
    wv1 = np.concatenate([Wv, bv[None, :]], 0).astype(np.float32)
    maps = []
    for core in range(NCORES):
        b, half = core // 2, core % 2
        x1t = np.ascontiguousarray(np.concatenate(
            [x[b], np.ones((L, 1), np.float32)], 1
        ).T.astype(np.float32))              # [65, L]
        xk = np.ascontiguousarray(x1t[:, half * KSH:(half + 1) * KSH])
        maps.append({
            "xt": x1t,
            "xk": xk,
            "wq": wq1, "wk": wk1, "wv": wv1,
        })
    return maps


def _run(x, Wq, bq, Wk, bk, Wv, bv, trace=False):
    from concourse.bass_utils import run_bass_kernel_spmd

    nc = _get_nc()
    maps = _in_maps(x, Wq, bq, Wk, bk, Wv, bv)
    res = run_bass_kernel_spmd(
        nc, maps, core_ids=list(range(NCORES)), trace=trace
    )
    outs = [r["o"].astype(np.float32) for r in res.results]
    full = np.empty((B, L, C), np.float32)
    for b in range(B):
        full[b] = outs[2 * b] + outs[2 * b + 1] + x[b]
    return full, res


def kernel(x, Wq, bq, Wk, bk, Wv, bv):
    x = np.asarray(x, np.float32)
    full, _ = _run(
        x,
        np.asarray(Wq, np.float32), np.asarray(bq, np.float32),
        np.asarray(Wk, np.float32), np.asarray(bk, np.float32),
        np.asarray(Wv, np.float32), np.asarray(bv, np.float32),
    )
    return full


# revision 9
# speedup vs baseline: 1.2941x; 1.2005x over previous
"""Trainium2 Bass kernel for nn_Attention1D (B=4, L=4096, C=64).

reference:
    Q = x@Wq + bq ; K = x@Wk + bk ; V = x@Wv + bv          (per batch b)
    s = Q @ K.T / sqrt(C)                                   [L_q, L_k]
    attn = softmax(s, axis=q)      # normalize over QUERY axis
    out = attn @ V + x

Sharding: 8 cores = 4 batches x 2 key-shards (k in [0,2048) / [2048,4096)).
The softmax normalizes over q, which is NOT sharded, so each core's softmax
is fully local:
    Z[k]   = sum_q exp(s[q,k])
    out_qf = sum_k exp(s[q,k]) * (V[k,f]/Z[k])
and the two k-shards' partial outputs simply ADD. The host sums the pair
and adds the residual x (the residual dominates the output, making the
attention path tolerant of bf16 everywhere: measured ~1e-3 rel err vs the
2e-2 gate).

Roofline: the ScalarE (ACT) exp of 2048x4096 = 8.4M score elements per
core is the binding engine: 64 x [128,1024] chunks at (1024+352)cyc/1.2GHz
plus ~190ns READ_ACCUMULATOR each ~= 84us. The whole structure keeps ACT
back-to-back:
  - everything bf16 (host casts x and weights): bf16 rhs streams 1
    col/cycle (fp32 is ~2x slower and its FP32-HI mode disables FWL for
    following LDWEIGHTS). AV LDWEIGHTS then hide under the matmuls.
  - a dummy exp at t=0 forces the ~1.3us ACT table load during the DMAs.
  - k-tiles processed SINGLY; each [128,1024] score chunk row-packs the
    SAME k-tile over two 512-q windows (tile_position (0,0)/(64,0), with
    K/Q rows duplicated host-side), so one chunk occupies ONE PSUM slot
    and the 2-slot rotation truly double-buffers: scores for chunk c+2
    run during exp(c+1), gated only on READ_ACC(c).
  - AV matmuls of tile t-1 (8 per chunk) + one small deferred projection
    item fill the PE slack between score pairs.
  - PSUM: 2 x [128,1024]f32 score slots (4 banks) + 4 x [128,8,64]f32 out
    accumulators (1 bank each; separate tiles so the tail evacuation of
    bank g doesn't false-dep the remaining AV matmuls).
    matmul start=True clears has_written for the WHOLE bank, so only the
    first accumulator chunk-MM per bank sets it.

Layout: channel-major (c on partitions), scores transposed sT[k, q] with
the softmax axis on the free dim. Host pre-transposes x and appends a
ones-row so biases ride inside the weights (contract dim 65); 1/sqrt(C)
is folded into Wq. No max-subtraction (|s| <= ~9, exp is safe in fp32).
"""

import numpy as np
import ml_dtypes

B, L, C = 4, 4096, 64
NCORES = 8
KSH = L // 2          # k columns per core: 2048
NKT = KSH // 128      # 16 k-tiles per core
NQC = L // 128        # 32 q-chunks of 128
NQ1 = L // 1024       # 4 q-chunks of 1024

_cache = {}


def _build():
    # NOTE: --enable-ldw-opt=true fails walrus codegen on the tile_position
    # score LDWs ("InstLdweights is not compatible with LDW optimization").
    import concourse.bacc as bacc
    import concourse.mybir as mybir
    import concourse.tile as tile
    from concourse.bass import _add_dep_helper

    bf16 = mybir.dt.bfloat16
    f32 = mybir.dt.float32
    AF = mybir.ActivationFunctionType
    AX = mybir.AxisListType

    nc = bacc.Bacc("TRN2", target_bir_lowering=False, debug=False)

    xt_d = nc.dram_tensor("xt", [C + 1, L], bf16, kind="ExternalInput")
    xk_d = nc.dram_tensor("xk", [C + 1, KSH], bf16, kind="ExternalInput")
    wq_d = nc.dram_tensor("wq", [C + 1, 2 * C], bf16, kind="ExternalInput")
    wk_d = nc.dram_tensor("wk", [C + 1, 2 * C], bf16, kind="ExternalInput")
    wv_d = nc.dram_tensor("wv", [C + 1, C], bf16, kind="ExternalInput")
    o_d = nc.dram_tensor("o", [L, C], f32, kind="ExternalOutput")

    with tile.TileContext(nc) as tc:
        with (
            tc.tile_pool(name="consts", bufs=1) as consts,
            tc.tile_pool(name="sb", bufs=1) as sb,
            tc.tile_pool(name="etp", bufs=3) as etp,
            tc.tile_pool(name="gvp", bufs=3) as gvp,
            tc.tile_pool(name="zpp", bufs=6) as zpp,
            tc.tile_pool(name="scp", bufs=2, space="PSUM") as scp,
            tc.tile_pool(name="accp", bufs=1, space="PSUM") as accp,
        ):
            # --- ACT table warmer: walrus inserts the ~1.3us
            # PSEUDO_LOAD_ACT_FUNC_SET before this dummy exp, so the table
            # is resident long before the first real score chunk. ---
            jk = consts.tile([128, 1], f32)
            nc.vector.memset(jk, 0.0)
            jko = consts.tile([128, 1], f32)
            nc.scalar.activation(out=jko, in_=jk, func=AF.Exp)

            # --- HAM warmup: dense dummy matmuls while the DMAs stream in ---
            wu = consts.tile([128, 512], bf16)
            nc.vector.memset(wu, 0.0)
            for _ in range(8):
                ps = scp.tile([128, 1024], f32, tag="s")
                nc.tensor.matmul(ps[:, 0:512], lhsT=wu[:, 0:128], rhs=wu,
                                 start=True, stop=True)

            # --- input DMAs, critical-path order (Sync queue serializes
            # issue at ~0.8us each): K path first, then Q chunk 0. ---
            wk_s = consts.tile([C + 1, 2 * C], bf16)
            wq_s = consts.tile([C + 1, 2 * C], bf16)
            wv_s = consts.tile([C + 1, C], bf16)
            xk_c, xt_c = [], []

            def dma_xk(c):
                t = sb.tile([C + 1, 1024], bf16, tag=f"xk{c}", name=f"xk{c}")
                nc.sync.dma_start(out=t, in_=xk_d.ap()[:, c * 1024:(c + 1) * 1024])
                xk_c.append(t)

            def dma_xt(c):
                t = sb.tile([C + 1, 1024], bf16, tag=f"xt{c}", name=f"xt{c}")
                nc.sync.dma_start(out=t, in_=xt_d.ap()[:, c * 1024:(c + 1) * 1024])
                xt_c.append(t)

            nc.sync.dma_start(out=wk_s, in_=wk_d.ap())
            dma_xk(0)
            nc.sync.dma_start(out=wq_s, in_=wq_d.ap())
            dma_xt(0)
            nc.sync.dma_start(out=wv_s, in_=wv_d.ap())
            dma_xt(1)
            dma_xt(2)
            dma_xt(3)
            dma_xk(1)

            # --- projections (bf16). QT/KT rows 0-63 and 64-127 hold the
            # SAME values (weights doubled host-side) for the row-packed
            # score matmuls. Emitted just-in-time: the prologue does KT
            # chunk 0 (k-tiles 0-7), QT chunk 0 and V0; the rest drain one
            # small item per main-loop chunk. ---
            kt_c = [sb.tile([128, 1024], bf16, tag=f"kt{c}", name=f"kt{c}")
                    for c in range(2)]
            qt_c = [sb.tile([128, 1024], bf16, tag=f"qt{c}", name=f"qt{c}")
                    for c in range(NQ1)]
            v_ts = [sb.tile([128, C], bf16, tag=f"v{kt}", name=f"v{kt}")
                    for kt in range(NKT)]

            def emit_kt(c, h):
                ps = scp.tile([128, 1024], f32, tag="s")
                nc.tensor.matmul(ps[:, h * 512:(h + 1) * 512], lhsT=wk_s,
                                 rhs=xk_c[c][:, h * 512:(h + 1) * 512],
                                 start=True, stop=True)
                nc.vector.tensor_copy(out=kt_c[c][:, h * 512:(h + 1) * 512],
                                      in_=ps[:, h * 512:(h + 1) * 512])

            def emit_qt(c, h):
                ps = scp.tile([128, 1024], f32, tag="s")
                nc.tensor.matmul(ps[:, h * 512:(h + 1) * 512], lhsT=wq_s,
                                 rhs=xt_c[c][:, h * 512:(h + 1) * 512],
                                 start=True, stop=True)
                nc.vector.tensor_copy(out=qt_c[c][:, h * 512:(h + 1) * 512],
                                      in_=ps[:, h * 512:(h + 1) * 512])

            def emit_v(kt):
                vps = scp.tile([128, 1024], f32, tag="s")
                nc.tensor.matmul(
                    vps[:, 0:C],
                    lhsT=xk_c[kt // 8][:, (kt % 8) * 128:(kt % 8 + 1) * 128],
                    rhs=wv_s, start=True, stop=True,
                )
                nc.vector.tensor_copy(out=v_ts[kt], in_=vps[:, 0:C])

            emit_kt(0, 0)
            emit_kt(0, 1)
            emit_qt(0, 0)
            emit_qt(0, 1)
            emit_qt(1, 0)
            emit_qt(1, 1)
            emit_v(0)

            # deferred projection work, drained into PE slack between score
            # chunks. qt_c[j] must be fully emitted before chunk (0, j)'s
            # score MMs (FIFO deadlock otherwise), so the first two chunks
            # pop two items each.
            deferred = []
            deferred.append(lambda: emit_qt(2, 0))
            deferred.append(lambda: emit_qt(2, 1))
            deferred.append(lambda: emit_qt(3, 0))
            deferred.append(lambda: emit_qt(3, 1))
            for k in range(1, 8):
                deferred.append(lambda k=k: emit_v(k))
            deferred.append(lambda: emit_kt(1, 0))
            deferred.append(lambda: emit_kt(1, 1))
            for k in range(8, NKT):
                deferred.append(lambda k=k: emit_v(k))
            pops = {(0, 0): 2, (0, 1): 2}

            # --- out accumulators: one tile per PSUM bank for precise
            # tail deps (evac of bank g vs AV matmuls of bank g') ---
            accs = [accp.tile([128, 8, C], f32, tag=f"acc{g}", name=f"acc{g}")
                    for g in range(4)]

            def emit_av_group(p, c2):
                # 8 AV chunk-MMs of the previous k-tile, interleaved between
                # score chunks to keep the PE dense.
                et_p, gv_p, kt_p = p
                for qc in range(c2 * 8, c2 * 8 + 8):
                    # start=True clears has_written for the WHOLE bank: only
                    # the first chunk-MM per bank may set it; later chunks
                    # overwrite-where-unset, which sets their own bits.
                    nc.tensor.matmul(
                        accs[c2][:, qc - c2 * 8, :],
                        lhsT=et_p[:, qc * 128:(qc + 1) * 128],
                        rhs=gv_p,
                        start=(kt_p == 0 and qc % 8 == 0),
                        stop=(kt_p == NKT - 1),
                        skip_group_check=True,
                    )

            # --- main loop over k-tiles (singly) ---
            # Per chunk (k-tile kt, q-window c2 of 1024): the two 512-q
            # halves co-issue via same-tile row packing (rows 0-63 / 64-127
            # both hold this k-tile's KT columns; QT rows duplicated).
            prev = None
            last = None
            for kt in range(NKT):
                et = etp.tile([128, L], bf16, tag="et")
                zp = zpp.tile([128, 4], f32, tag="zp")
                lA = kt_c[kt // 8][0:C, (kt % 8) * 128:(kt % 8 + 1) * 128]
                lB = kt_c[kt // 8][C:128, (kt % 8) * 128:(kt % 8 + 1) * 128]
                for c2 in range(4):
                    st = scp.tile([128, 1024], f32, tag="s")
                    rhs = qt_c[c2]
                    ma = nc.tensor.matmul(
                        st[:, 0:512], lhsT=lA, rhs=rhs[0:C, 0:512],
                        tile_position=(0, 0), start=True, stop=True,
                    )
                    mb = nc.tensor.matmul(
                        st[:, 512:1024], lhsT=lB, rhs=rhs[C:128, 512:1024],
                        tile_position=(C, 0), start=True, stop=True,
                    )
                    # keep the two halves adjacent in the static PE order so
                    # they co-issue (row packing)
                    if last is not None:
                        _add_dep_helper(ma.ins, last.ins, sync=False,
                                        reason="pair order")
                    _add_dep_helper(mb.ins, ma.ins, sync=False,
                                    reason="pair order")
                    last = mb
                    nc.scalar.activation(
                        out=et[:, c2 * 1024:(c2 + 1) * 1024], in_=st,
                        func=AF.Exp, accum_out=zp[:, c2:c2 + 1],
                    )
                    if prev is not None:
                        emit_av_group(prev, c2)
                    for _ in range(pops.get((kt, c2), 1)):
                        if deferred:
                            deferred.pop(0)()
                z = zpp.tile([128, 1], f32, tag="z")
                nc.vector.reduce_sum(out=z, in_=zp, axis=AX.X)
                rz = zpp.tile([128, 1], f32, tag="rz")
                nc.vector.reciprocal(out=rz, in_=z)
                gv = gvp.tile([128, C], bf16, tag="gv")
                nc.vector.tensor_scalar_mul(gv, v_ts[kt], rz)
                prev = (et, gv, kt)
            # final tile's AV drain, interleaved with the per-bank
            # evacuation + store so the tail overlaps the remaining AV work
            o_ap = o_d.ap()
            for g in range(4):
                emit_av_group(prev, g)
                ob = sb.tile([128, 8, C], f32, tag=f"ob{g}", name=f"ob{g}")
                nc.vector.tensor_copy(out=ob, in_=accs[g])
                nc.sync.dma_start(
                    out=o_ap[g * 1024:(g + 1) * 1024, :].rearrange(
                        "(t p) f -> p t f", p=128
                    ),
                    in_=ob,
                )

    nc.compile()
    return nc


def _get_nc():
    if "nc" not in _cache:
        _cache["nc"] = _build()
    return _cache["nc"]


def _in_maps(x, Wq, bq, Wk, bk, Wv, bv):
    bf = ml_dtypes.bfloat16
    s = 1.0 / np.sqrt(np.float32(C))
    wq1 = (np.concatenate([Wq, bq[None, :]], 0) * s).astype(np.float32)
    wq1 = np.concatenate([wq1, wq1], 1).astype(bf)   # doubled -> replicated QT
    wk1 = np.concatenate([Wk, bk[None, :]], 0).astype(np.float32)
    wk1 = np.concatenate([wk1, wk1], 1).astype(bf)
    wv1 = np.concatenate([Wv, bv[None, :]], 0).astype(bf)
    maps = []
    for core in range(NCORES):
        b, half = core // 2, core % 2
        x1t = np.ascontiguousarray(np.concatenate(
            [x[b], np.ones((L, 1), np.float32)], 1
        ).T.astype(bf))                      # [65, L]
        xk = np.ascontiguousarray(x1t[:, half * KSH:(half + 1) * KSH])
        maps.append({
            "xt": x1t,
            "xk": xk,
            "wq": wq1, "wk": wk1, "wv": wv1,
        })
    return maps


def _run(x, Wq, bq, Wk, bk, Wv, bv, trace=False):
    from concourse.bass_utils import run_bass_kernel_spmd

    nc = _get_nc()
    maps = _in_maps(x, Wq, bq, Wk, bk, Wv, bv)
    res = run_bass_kernel_spmd(
        nc, maps, core_ids=list(range(NCORES)), trace=trace
    )
    outs = [r["o"].astype(np.float32) for r in res.results]
    full = np.empty((B, L, C), np.float32)
    for b in range(B):
        full[b] = outs[2 * b] + outs[2 * b + 1] + x[b]
    return full, res


def kernel(x, Wq, bq, Wk, bk, Wv, bv):
    x = np.asarray(x, np.float32)
    full, _ = _run(
        x,
        np.asarray(Wq, np.float32), np.asarray(bq, np.float32),
        np.asarray(Wk, np.float32), np.asarray(bk, np.float32),
        np.asarray(Wv, np.float32), np.asarray(bv, np.float32),
    )
    return full


# revision 11
# speedup vs baseline: 1.3310x; 1.0285x over previous
"""Trainium2 Bass kernel for nn_Attention1D (B=4, L=4096, C=64).

reference:
    Q = x@Wq + bq ; K = x@Wk + bk ; V = x@Wv + bv          (per batch b)
    s = Q @ K.T / sqrt(C)                                   [L_q, L_k]
    attn = softmax(s, axis=q)      # normalize over QUERY axis
    out = attn @ V + x

Sharding: 8 cores = 4 batches x 2 key-shards (k in [0,2048) / [2048,4096)).
The softmax normalizes over q, which is NOT sharded, so each core's softmax
is fully local:
    Z[k]   = sum_q exp(s[q,k])
    out_qf = sum_k exp(s[q,k]) * (V[k,f]/Z[k])
and the two k-shards' partial outputs simply ADD. The host sums the pair
and adds the residual x (the residual dominates the output, making the
attention path tolerant of bf16 everywhere: ~1e-3 rel err vs the 2e-2
gate).

Roofline: the ScalarE (ACT) exp of 2048x4096 = 8.4M score elements per
core is the binding engine: 64 x [128,1024] chunks at ~1.11us ACTIVATE +
~0.19us READ_ACCUMULATOR each ~= 84us. The whole structure keeps ACT
back-to-back:
  - everything bf16 (host casts x and weights): bf16 rhs streams 1
    col/cycle (fp32 is ~2x slower and its FP32-HI mode disables FWL for
    following LDWEIGHTS). AV LDWEIGHTS then hide under the matmuls
    (measured 53ns/AV-matmul pitch).
  - a dummy exp at t=0 forces the ~1.3us ACT table load during the DMAs.
  - k-tiles processed SINGLY; each [128,1024] score chunk row-packs the
    SAME k-tile over two 512-q windows (tile_position (0,0)/(64,0), with
    K/Q rows duplicated host-side), so one chunk occupies ONE PSUM slot
    and the 2-slot rotation truly double-buffers: scores for chunk c+2
    run during exp(c+1), gated only on READ_ACC(c).
  - AV matmul groups run SLID by 6 chunks (unit j at chunk j+6) so the
    Z->reciprocal->GV DVE chain of a tile never blocks the PE FIFO.
  - projections are batched (qt/kt chunk = 2 MMs in one PSUM slot, V in
    groups of 4 tiles per slot) because every projection steals a score-
    slot rotation, putting the next score matmul 1-apart (instead of
    2-apart) from the exp it WARs against. 8 steals total, all in the
    first ~6 chunks.
  - PSUM: 2 x [128,1024]f32 score slots (4 banks) + 4 x [128,8,64]f32 out
    accumulators (1 bank each; separate tiles so the tail evacuation of
    bank g doesn't false-dep the remaining AV matmuls).
    matmul start=True clears has_written for the WHOLE bank, so only the
    first write to a bank may set it.
  - output is stored partition-major [128, 32, 64] (contiguous 2KB per
    partition DMA); the host un-permutes, which is free next to the
    host-side shard-sum + residual add.

Layout: channel-major (c on partitions), scores transposed sT[k, q] with
the softmax axis on the free dim. Host pre-transposes x and appends a
ones-row so biases ride inside the weights (contract dim 65); 1/sqrt(C)
is folded into Wq. No max-subtraction (|s| <= ~9, exp is safe in fp32).
"""

import numpy as np
import ml_dtypes

B, L, C = 4, 4096, 64
NCORES = 8
KSH = L // 2          # k columns per core: 2048
NKT = KSH // 128      # 16 k-tiles per core
NQC = L // 128        # 32 q-chunks of 128
NQ1 = L // 1024       # 4 q-chunks of 1024
SLIDE = 6             # AV unit j runs at chunk j+SLIDE

_cache = {}


def _build():
    # NOTE: --enable-ldw-opt=true fails walrus codegen on the tile_position
    # score LDWs ("InstLdweights is not compatible with LDW optimization").
    import concourse.bacc as bacc
    import concourse.mybir as mybir
    import concourse.tile as tile
    from concourse.bass import _add_dep_helper

    bf16 = mybir.dt.bfloat16
    f32 = mybir.dt.float32
    AF = mybir.ActivationFunctionType
    AX = mybir.AxisListType

    nc = bacc.Bacc("TRN2", target_bir_lowering=False, debug=False)

    xt_d = nc.dram_tensor("xt", [C + 1, L], bf16, kind="ExternalInput")
    xk_d = nc.dram_tensor("xk", [C + 1, KSH], bf16, kind="ExternalInput")
    wq_d = nc.dram_tensor("wq", [C + 1, 2 * C], bf16, kind="ExternalInput")
    wk_d = nc.dram_tensor("wk", [C + 1, 2 * C], bf16, kind="ExternalInput")
    wv_d = nc.dram_tensor("wv", [C + 1, C], bf16, kind="ExternalInput")
    o_d = nc.dram_tensor("o", [128, NQC, C], f32, kind="ExternalOutput")

    with tile.TileContext(nc) as tc:
        with (
            tc.tile_pool(name="consts", bufs=1) as consts,
            tc.tile_pool(name="sb", bufs=1) as sb,
            tc.tile_pool(name="etp", bufs=3) as etp,
            tc.tile_pool(name="gvp", bufs=3) as gvp,
            tc.tile_pool(name="zpp", bufs=6) as zpp,
            tc.tile_pool(name="scp", bufs=2, space="PSUM") as scp,
            tc.tile_pool(name="accp", bufs=1, space="PSUM") as accp,
        ):
            # --- ACT table warmer: walrus inserts the ~1.3us
            # PSEUDO_LOAD_ACT_FUNC_SET before this dummy exp, so the table
            # is resident long before the first real score chunk. ---
            jk = consts.tile([128, 1], f32)
            nc.vector.memset(jk, 0.0)
            jko = consts.tile([128, 1], f32)
            nc.scalar.activation(out=jko, in_=jk, func=AF.Exp)

            # --- input DMAs, critical-path order (Sync queue serializes
            # issue at ~0.8us each): K path first, then Q chunk 0. ---
            wk_s = consts.tile([C + 1, 2 * C], bf16)
            wq_s = consts.tile([C + 1, 2 * C], bf16)
            wv_s = consts.tile([C + 1, C], bf16)
            xk_c, xt_c = [], []

            def dma_xk(c):
                t = sb.tile([C + 1, 1024], bf16, tag=f"xk{c}", name=f"xk{c}")
                nc.sync.dma_start(out=t, in_=xk_d.ap()[:, c * 1024:(c + 1) * 1024])
                xk_c.append(t)

            def dma_xt(c):
                t = sb.tile([C + 1, 1024], bf16, tag=f"xt{c}", name=f"xt{c}")
                nc.sync.dma_start(out=t, in_=xt_d.ap()[:, c * 1024:(c + 1) * 1024])
                xt_c.append(t)

            nc.sync.dma_start(out=wk_s, in_=wk_d.ap())
            dma_xk(0)
            nc.sync.dma_start(out=wq_s, in_=wq_d.ap())
            dma_xt(0)
            nc.sync.dma_start(out=wv_s, in_=wv_d.ap())
            dma_xt(1)
            dma_xt(2)
            dma_xt(3)
            dma_xk(1)

            # --- projections (bf16). QT/KT rows 0-63 and 64-127 hold the
            # SAME values (weights doubled host-side) for the row-packed
            # score matmuls. Each emission batches all its matmuls into ONE
            # score-slot rotation. The prologue does only KT chunk 0
            # (k-tiles 0-7) and QT chunk 0; the rest drain into scheduled
            # slots in the first ~6 main-loop chunks. ---
            kt_c = [sb.tile([128, 1024], bf16, tag=f"kt{c}", name=f"kt{c}")
                    for c in range(2)]
            qt_c = [sb.tile([128, 1024], bf16, tag=f"qt{c}", name=f"qt{c}")
                    for c in range(NQ1)]
            v4_ts = [sb.tile([128, 4, C], bf16, tag=f"v4_{g}", name=f"v4_{g}")
                     for g in range(4)]

            def emit_kt(c):
                ps = scp.tile([128, 1024], f32, tag="s")
                for h in range(2):
                    nc.tensor.matmul(ps[:, h * 512:(h + 1) * 512], lhsT=wk_s,
                                     rhs=xk_c[c][:, h * 512:(h + 1) * 512],
                                     start=True, stop=True)
                nc.vector.tensor_copy(out=kt_c[c], in_=ps)

            def emit_qt(c):
                ps = scp.tile([128, 1024], f32, tag="s")
                for h in range(2):
                    nc.tensor.matmul(ps[:, h * 512:(h + 1) * 512], lhsT=wq_s,
                                     rhs=xt_c[c][:, h * 512:(h + 1) * 512],
                                     start=True, stop=True)
                nc.vector.tensor_copy(out=qt_c[c], in_=ps)

            def emit_v4(g):
                # 4 V k-tiles into one slot (one bank): only the first MM
                # sets has_written for the bank (whole-bank clear rule).
                vps = scp.tile([128, 1024], f32, tag="s")
                for i in range(4):
                    kt = g * 4 + i
                    nc.tensor.matmul(
                        vps[:, i * C:(i + 1) * C],
                        lhsT=xk_c[kt // 8][:, (kt % 8) * 128:(kt % 8 + 1) * 128],
                        rhs=wv_s, start=(i == 0), stop=(i == 3),
                        skip_group_check=True,
                    )
                nc.vector.tensor_copy(
                    out=v4_ts[g], in_=vps[:, 0:4 * C].rearrange(
                        "p (t f) -> p t f", t=4))

            emit_kt(0)
            emit_qt(0)

            # deferred projections: chunk index -> list of emits. qt_c[j]
            # must be fully emitted before chunk (0, j)'s score MMs (FIFO
            # deadlock otherwise). v4 group g feeds gv of tiles 4g..4g+3.
            deferred = {
                0: [lambda: emit_qt(1)],
                1: [lambda: emit_qt(2), lambda: emit_v4(0)],
                2: [lambda: emit_qt(3)],
                3: [lambda: emit_v4(1)],
                4: [lambda: emit_kt(1)],
                5: [lambda: emit_v4(2)],
                6: [lambda: emit_v4(3)],
            }

            # --- out accumulators: one tile per PSUM bank for precise
            # tail deps (evac of bank g doesn't block AV of bank g') ---
            accs = [accp.tile([128, 8, C], f32, tag=f"acc{g}", name=f"acc{g}")
                    for g in range(4)]

            gvs = [None] * NKT
            ets = [None] * NKT

            def emit_av_unit(j):
                # 8 AV chunk-MMs: tile j//4 into acc bank j%4.
                kt_p, bank = j // 4, j % 4
                et_p, gv_p = ets[kt_p], gvs[kt_p]
                for qc in range(bank * 8, bank * 8 + 8):
                    nc.tensor.matmul(
                        accs[bank][:, qc - bank * 8, :],
                        lhsT=et_p[:, qc * 128:(qc + 1) * 128],
                        rhs=gv_p,
                        start=(kt_p == 0 and qc % 8 == 0),
                        stop=(kt_p == NKT - 1),
                        skip_group_check=True,
                    )

            # --- main loop over k-tiles (singly) ---
            # Per chunk (k-tile kt, q-window c2 of 1024): the two 512-q
            # halves co-issue via same-tile row packing (rows 0-63 / 64-127
            # both hold this k-tile's KT columns; QT rows duplicated).
            last = None
            for kt in range(NKT):
                et = etp.tile([128, L], bf16, tag="et")
                ets[kt] = et
                zp = zpp.tile([128, 4], f32, tag="zp")
                lA = kt_c[kt // 8][0:C, (kt % 8) * 128:(kt % 8 + 1) * 128]
                lB = kt_c[kt // 8][C:128, (kt % 8) * 128:(kt % 8 + 1) * 128]
                for c2 in range(4):
                    g = kt * 4 + c2
                    st = scp.tile([128, 1024], f32, tag="s")
                    rhs = qt_c[c2]
                    ma = nc.tensor.matmul(
                        st[:, 0:512], lhsT=lA, rhs=rhs[0:C, 0:512],
                        tile_position=(0, 0), start=True, stop=True,
                    )
                    mb = nc.tensor.matmul(
                        st[:, 512:1024], lhsT=lB, rhs=rhs[C:128, 512:1024],
                        tile_position=(C, 0), start=True, stop=True,
                    )
                    # keep the two halves adjacent in the static PE order so
                    # they co-issue (row packing)
                    if last is not None:
                        _add_dep_helper(ma.ins, last.ins, sync=False,
                                        reason="pair order")
                    _add_dep_helper(mb.ins, ma.ins, sync=False,
                                    reason="pair order")
                    last = mb
                    nc.scalar.activation(
                        out=et[:, c2 * 1024:(c2 + 1) * 1024], in_=st,
                        func=AF.Exp, accum_out=zp[:, c2:c2 + 1],
                    )
                    if g - SLIDE >= 0:
                        emit_av_unit(g - SLIDE)
                    for fn in deferred.pop(g, ()):
                        fn()
                z = zpp.tile([128, 1], f32, tag="z")
                nc.vector.reduce_sum(out=z, in_=zp, axis=AX.X)
                rz = zpp.tile([128, 1], f32, tag="rz")
                nc.vector.reciprocal(out=rz, in_=z)
                gv = gvp.tile([128, C], bf16, tag="gv")
                nc.vector.tensor_scalar_mul(gv, v4_ts[kt // 4][:, kt % 4, :], rz)
                gvs[kt] = gv
            # tail: remaining AV units, evacuation of bank g interleaved
            # right after its last AV unit
            o_ap = o_d.ap()
            for j in range(4 * NKT - SLIDE, 4 * NKT):
                emit_av_unit(j)
                bank = j % 4
                if j // 4 == NKT - 1:
                    ob = sb.tile([128, 8, C], f32, tag=f"ob{bank}",
                                 name=f"ob{bank}")
                    nc.vector.tensor_copy(out=ob, in_=accs[bank])
                    nc.sync.dma_start(
                        out=o_ap[:, bank * 8:(bank + 1) * 8, :], in_=ob)

    nc.compile()
    return nc


def _get_nc():
    if "nc" not in _cache:
        _cache["nc"] = _build()
    return _cache["nc"]


def _in_maps(x, Wq, bq, Wk, bk, Wv, bv):
    bf = ml_dtypes.bfloat16
    s = 1.0 / np.sqrt(np.float32(C))
    wq1 = (np.concatenate([Wq, bq[None, :]], 0) * s).astype(np.float32)
    wq1 = np.concatenate([wq1, wq1], 1).astype(bf)   # doubled -> replicated QT
    wk1 = np.concatenate([Wk, bk[None, :]], 0).astype(np.float32)
    wk1 = np.concatenate([wk1, wk1], 1).astype(bf)
    wv1 = np.concatenate([Wv, bv[None, :]], 0).astype(bf)
    maps = []
    for core in range(NCORES):
        b, half = core // 2, core % 2
        x1t = np.ascontiguousarray(np.concatenate(
            [x[b], np.ones((L, 1), np.float32)], 1
        ).T.astype(bf))                      # [65, L]
        xk = np.ascontiguousarray(x1t[:, half * KSH:(half + 1) * KSH])
        maps.append({
            "xt": x1t,
            "xk": xk,
            "wq": wq1, "wk": wk1, "wv": wv1,
        })
    return maps


def _assemble(results, x):
    # device output is partition-major [128, 32, 64]: out[t*128+p] = o[p, t]
    outs = [
        r["o"].astype(np.float32).transpose(1, 0, 2).reshape(L, C)
        for r in results
    ]
    full = np.empty((B, L, C), np.float32)
    for b in range(B):
        full[b] = outs[2 * b] + outs[2 * b + 1] + x[b]
    return full


def _run(x, Wq, bq, Wk, bk, Wv, bv, trace=False):
    from concourse.bass_utils import run_bass_kernel_spmd

    nc = _get_nc()
    maps = _in_maps(x, Wq, bq, Wk, bk, Wv, bv)
    res = run_bass_kernel_spmd(
        nc, maps, core_ids=list(range(NCORES)), trace=trace
    )
    return _assemble(res.results, x), res


def kernel(x, Wq, bq, Wk, bk, Wv, bv):
    x = np.asarray(x, np.float32)
    full, _ = _run(
        x,
        np.asarray(Wq, np.float32), np.asarray(bq, np.float32),
        np.asarray(Wk, np.float32), np.asarray(bk, np.float32),
        np.asarray(Wv, np.float32), np.asarray(bv, np.float32),
    )
    return full
